# revision 22
# baseline (speedup 1.0000x reference)
"""Fused RoBERTa layer (attention + FFN, LoRA merged) on 8 Trainium2 cores.

Sharding: pure data-parallel over batch (16 batches -> 2 per core), no
collectives. LoRA merged into base weights on host; 1/sqrt(hd) folded into
w_q.

fp8 strategy (2x PE throughput via DoubleRow double-pumping):
  - QKV / AV / O-proj / FFN-up matmuls run fp8e4m3 with
    MatmulPerfMode.DoubleRow: both operands hold TWO 128-deep K-chunks side
    by side in the free dim ([128, 2, M]), contracting 256 per pass.
  - FFN-down stays bf16 (fp8 there breaks the accuracy gate).
  - Weights pre-scaled by 2^7 (2^10 for w_q) into fp8 normal range; inverse
    scales folded into exp input scale (2^-17), gelu input scale (2^-7),
    V-evict scale (2^-7), and a host 2^7 pre-scale of the bf16 residual x
    (LN1 is scale-invariant).
  - exp folds a 2^-9 output scale via its bias so unnormalized fp8
    attention weights stay in range; the ones-column denominator is the sum
    of the SAME fp8 weights so the scale cancels exactly.

Attention normalization is BATCHED: the AV matmul emits unnormalized o
rows plus a denominator row per (b,h) (V' ones-column; for odd heads the
ones column comes FIRST and the AV output is written at partition offset
63 so o rows land on partitions 64..127 -- this keeps every evict
same-partition and vector-legal). Unnormalized o is evicted to bf16 oTu;
denominators collect into a [H, T] tile; per batch one reciprocal + one
DMA out + 8 broadcast DMAs back + 8 vector muls produce fp8 oT. The
post-pass of batch b is interleaved into batch b+1's attention (or the
O-projection) so its DMA latency is hidden.

PSUM: one 8-bank pool with manual tags: mm0/mm1 (projection/FFN double
buffer), sc0..3 (scores 4-deep, reused by the LN1 transposes), pv0/pv1
(AV). Host pre-swizzles all fp8 tensors into per-tile [128, W] contiguous
layout so every weight DMA is linear in DRAM.
"""

import math
import sys

sys.path.insert(0, "/opt/trn_rl_repo")

import numpy as np
import ml_dtypes

import concourse.bacc as bacc
import concourse.bass as bass
import concourse.tile as tile
from concourse import mybir
from concourse.bass_utils import run_bass_kernel_spmd
from concourse.masks import make_identity

BF16 = mybir.dt.bfloat16
FP8 = mybir.dt.float8e4
F32 = mybir.dt.float32
NP_BF16 = np.dtype(ml_dtypes.bfloat16)
NP_FP8 = np.dtype(ml_dtypes.float8_e4m3)

B, S, D, H, HD, F = 16, 512, 1024, 16, 64, 4096
N_CORES = 8
TB = B // N_CORES
T = TB * S

MM_N = 512
P = 128

WSHIFT = 7
WS = float(2.0 ** WSHIFT)
QSHIFT = 10
QS = float(2.0 ** QSHIFT)
EXP_SCALE = float(2.0 ** (-(WSHIFT + QSHIFT)))
ATT_BIAS = -9 * math.log(2.0)


def _ceil_div(a, b):
    return (a + b - 1) // b


def build_program(cfg):
    D_, F_, T_, TB_, H_, HD_ = (cfg["D"], cfg["F"], cfg["T"], cfg["TB"],
                                cfg["H"], cfg["HD"])
    S_ = T_ // TB_
    KD = D_ // P
    KD2 = KD // 2
    KF = F_ // P
    TCH = T_ // P
    NT = _ceil_div(T_, MM_N)
    NTW = min(MM_N, T_)
    ND = _ceil_div(D_, MM_N)
    NDW = min(MM_N, D_)
    SKC = S_ // P
    SKC2 = SKC // 2
    HPC = P // HD_             # heads per 128-partition chunk (=2)
    VW = HD_ + 1               # V' per-head width (ones column)
    VROW = H_ * VW             # V' row width for one key chunk
    UPW = 1024
    UPT = F_ // UPW

    nc = bacc.Bacc("TRN2", target_bir_lowering=False, debug=False,
                   num_devices=N_CORES)

    # ---- DRAM I/O (fp8 tensors pre-swizzled on host: [ntile, 128, W]) ----
    xT8_d = nc.dram_tensor("xT8", [KD2, P, 2 * T_], FP8,
                           kind="ExternalInput")
    xr_d = nc.dram_tensor("xr", [T_, D_], BF16, kind="ExternalInput")
    wq_d = nc.dram_tensor("wq", [KD2, P, 2 * D_], FP8, kind="ExternalInput")
    wk_d = nc.dram_tensor("wk", [KD2, P, 2 * D_], FP8, kind="ExternalInput")
    wv_d = nc.dram_tensor("wv", [KD2, P, 2 * D_], FP8, kind="ExternalInput")
    wo_d = nc.dram_tensor("wo", [KD2, P, 2 * D_], FP8, kind="ExternalInput")
    wup_d = nc.dram_tensor("wup", [KD2 * UPT, P, 2 * UPW], FP8,
                           kind="ExternalInput")
    wdn_d = nc.dram_tensor("wdn", [F_, D_], BF16, kind="ExternalInput")
    bq_d = nc.dram_tensor("bq", [D_], F32, kind="ExternalInput")
    bk_d = nc.dram_tensor("bk", [D_], F32, kind="ExternalInput")
    bup_d = nc.dram_tensor("bup", [F_], F32, kind="ExternalInput")
    mask_d = nc.dram_tensor("maskT", [TB_, S_], F32, kind="ExternalInput")
    bv_d = nc.dram_tensor("bv", [D_], F32, kind="ExternalInput")
    bo_d = nc.dram_tensor("bo", [D_], F32, kind="ExternalInput")
    bdn_d = nc.dram_tensor("bdn", [D_], F32, kind="ExternalInput")
    g1_d = nc.dram_tensor("g1", [D_], F32, kind="ExternalInput")
    b1_d = nc.dram_tensor("b1", [D_], F32, kind="ExternalInput")
    g2_d = nc.dram_tensor("g2", [D_], F32, kind="ExternalInput")
    b2_d = nc.dram_tensor("b2", [D_], F32, kind="ExternalInput")
    out_d = nc.dram_tensor("out", [T_, D_], F32, kind="ExternalOutput")

    DR = mybir.MatmulPerfMode.DoubleRow

    with tile.TileContext(nc) as tc, \
         tc.tile_pool(name="consts", bufs=1) as consts, \
         tc.tile_pool(name="slab", bufs=1) as slab, \
         tc.tile_pool(name="pall", bufs=1, space="PSUM") as pall, \
         tc.tile_pool(name="work", bufs=2) as work, \
         tc.tile_pool(name="xrp", bufs=2) as xrp, \
         tc.tile_pool(name="attnp", bufs=6) as attnp, \
         tc.tile_pool(name="attn2", bufs=4) as attn2, \
         tc.tile_pool(name="rbp", bufs=1) as rbp, \
         tc.tile_pool(name="statp", bufs=4) as statp, \
         tc.tile_pool(name="outp", bufs=2) as outp, \
         tc.tile_pool(name="dramp", bufs=2, space="DRAM") as dramp:

        dma = nc.sync          # bulk loads
        dma2 = nc.gpsimd       # latency-bound small DMAs + output

        def slot(tag, width, dtype):
            return slab.tile([P, width], dtype, tag=tag, name=f"t_{tag}")

        def pair(ap_2d, i2):
            return ap_2d.rearrange("p (i w) -> p i w", i=2) if i2 is None \
                else ap_2d.rearrange("p (i w) -> p i w", i=2)[:, :, i2]

        mm_ctr = [0]

        def mm_tile():
            mm_ctr[0] ^= 1
            return pall.tile([P, MM_N], F32, tag=f"mm{mm_ctr[0]}",
                             name="mm")

        # ---- constants ----
        eps_t = consts.tile([P, 1], F32)
        nc.vector.memset(eps_t, 1e-5)
        attb_t = consts.tile([P, 1], F32)
        nc.vector.memset(attb_t, ATT_BIAS)
        zero_t = consts.tile([P, 1], F32)
        nc.vector.memset(zero_t, 0.0)
        ident = consts.tile([P, P], BF16)
        make_identity(nc, ident)
        if cfg["has_bq"]:
            bq_sb = consts.tile([P, KD], F32)
            dma.dma_start(out=bq_sb,
                          in_=bq_d.ap().rearrange("(m p) -> p m", p=P))
        if cfg["has_bk"]:
            bk_sb = consts.tile([P, KD], F32)
            dma.dma_start(out=bk_sb,
                          in_=bk_d.ap().rearrange("(m p) -> p m", p=P))
        if cfg["has_bup"]:
            bup_sb = consts.tile([P, KF], F32)
            dma.dma_start(out=bup_sb,
                          in_=bup_d.ap().rearrange("(m p) -> p m", p=P))
        if cfg["has_mask"]:
            mask_sb = consts.tile([P, TB_ * SKC], F32)
            dma.dma_start(out=mask_sb,
                          in_=mask_d.ap().rearrange("b (kc p) -> p (b kc)",
                                                    p=P))
            mask2_sb = consts.tile([P, TB_ * SKC], F32)
            nc.vector.tensor_scalar_add(out=mask2_sb, in0=mask_sb,
                                        scalar1=ATT_BIAS)

        def bcast_row(dram_vec, n):
            t = consts.tile([P, n], F32, name=f"bc_{dram_vec.name}")
            dma.dma_start(out=t,
                          in_=dram_vec.ap().unsqueeze(0).to_broadcast([P, n]))
            return t

        bv_bc = bcast_row(bv_d, D_) if cfg["has_bv"] else None
        bo_bc = bcast_row(bo_d, D_) if cfg["has_bo"] else None
        bdn_bc = bcast_row(bdn_d, D_) if cfg["has_bdn"] else None
        g1_bc = bcast_row(g1_d, D_) if cfg["has_n1"] else None
        b1_bc = bcast_row(b1_d, D_) if cfg["has_n1"] else None
        g2_bc = bcast_row(g2_d, D_) if cfg["has_n2"] else None
        b2_bc = bcast_row(b2_d, D_) if cfg["has_n2"] else None

        # ---- load x^T and QKV weights (fp8, K-paired, linear DMAs) ----
        xT8_sb = [slot(f"xT8{c2}", 2 * T_, FP8) for c2 in range(KD2)]
        w_sb = {nm: [slot(f"w{nm}{c2}", 2 * D_, FP8) for c2 in range(KD2)]
                for nm in ("q", "k", "v")}
        dma3 = nc.scalar       # second DMA queue for the cold start
        for c2 in range(KD2):
            dma.dma_start(out=w_sb["q"][c2], in_=wq_d[c2])
            dma3.dma_start(out=xT8_sb[c2], in_=xT8_d[c2])
        for c2 in range(KD2):
            dma3.dma_start(out=w_sb["k"][c2], in_=wk_d[c2])
        for c2 in range(KD2):
            dma.dma_start(out=w_sb["v"][c2], in_=wv_d[c2])

        qT_sb = [slot(f"qT{c}", T_, BF16) for c in range(KD)]
        kTe_sb = [slot(f"kTe{c}", T_, BF16) for c in range(KD)]
        kTo_sb = [slot(f"kTo{c}", T_, BF16) for c in range(KD)]
        for c in range(KD):
            nc.gpsimd.memset(kTe_sb[c][P // 2:P, :], 0.0)
            nc.gpsimd.memset(kTo_sb[c][0:P // 2, :], 0.0)
        Vp8_sb = [slot(f"Vp{c}", 2 * VROW, FP8) for c in range(TCH // 2)]

        # ---- QKV projections (fp8 DoubleRow) ----
        # t2=1 (second batch) halves are deferred into the attention phase
        # as PE filler while the ScalarE runs softmax exps.
        HB = P // 2

        def qk_proj(nm, m, t2):
            has_b = cfg["has_bq"] if nm == "q" else cfg["has_bk"]
            bias = (bq_sb if nm == "q" else bk_sb) if has_b else None
            if True:
                if True:
                    pt = mm_tile()
                    for c2 in range(KD2):
                        nc.tensor.matmul(
                            pt[:, :NTW],
                            lhsT=pair(w_sb[nm][c2],
                                      slice(m * P, (m + 1) * P)),
                            rhs=pair(xT8_sb[c2],
                                     slice(t2 * MM_N, t2 * MM_N + NTW)),
                            start=(c2 == 0), stop=(c2 == KD2 - 1),
                            perf_mode=DR)
                    sl = slice(t2 * MM_N, t2 * MM_N + NTW)
                    if nm == "q":
                        if has_b:
                            nc.vector.tensor_scalar_add(
                                out=qT_sb[m][:, sl],
                                in0=pt[:, :NTW], scalar1=bias[:, m:m + 1])
                        else:
                            nc.vector.tensor_copy(out=qT_sb[m][:, sl],
                                                  in_=pt[:, :NTW])
                    else:
                        if has_b:
                            nc.vector.tensor_scalar_add(
                                out=kTe_sb[m][0:HB, sl],
                                in0=pt[0:HB, :NTW],
                                scalar1=bias[0:HB, m:m + 1])
                            nc.vector.tensor_scalar_add(
                                out=kTo_sb[m][HB:P, sl],
                                in0=pt[HB:P, :NTW],
                                scalar1=bias[HB:P, m:m + 1])
                        else:
                            nc.vector.tensor_copy(out=kTe_sb[m][0:HB, sl],
                                                  in_=pt[0:HB, :NTW])
                            nc.vector.tensor_copy(out=kTo_sb[m][HB:P, sl],
                                                  in_=pt[HB:P, :NTW])

        for nm in ("q", "k"):
            for m in range(KD):
                qk_proj(nm, m, 0)
        # V token-major into V' ([v(64), 1] per head; 2^-7 scale on evict)
        def v_proj_tr(tr):
            vdst = Vp8_sb[tr // 2][:, (tr % 2) * VROW:(tr % 2 + 1) * VROW]
            vd3 = vdst.rearrange("p (h c) -> p h c", c=VW)
            for n2 in range(ND):
                pt = mm_tile()
                for c2 in range(KD2):
                    nc.tensor.matmul(
                        pt[:, :NDW],
                        lhsT=pair(xT8_sb[c2], slice(tr * P, (tr + 1) * P)),
                        rhs=pair(w_sb["v"][c2],
                                 slice(n2 * MM_N, n2 * MM_N + NDW)),
                        start=(c2 == 0), stop=(c2 == KD2 - 1),
                        perf_mode=DR)
                hpn = NDW // HD_   # heads per N tile
                src = pt[:, :NDW].rearrange("p (h c) -> p h c", c=HD_)
                if cfg["has_bv"]:
                    tmp = work.tile([P, NDW], F32, tag="vtmp", name="vtmp")
                    nc.vector.tensor_add(
                        out=tmp, in0=pt[:, :NDW],
                        in1=bv_bc[:, n2 * MM_N:n2 * MM_N + NDW])
                    src = tmp.rearrange("p (h c) -> p h c", c=HD_)
                nc.vector.tensor_scalar_mul(
                    out=vd3[:, n2 * hpn:(n2 + 1) * hpn, 0:HD_], in0=src,
                    scalar1=1.0 / WS)
            nc.vector.memset(vd3[:, :, HD_:VW], 1.0)  # ones cols

        for tr in range(TCH // 2):   # b0 chunks now; rest inside attention
            v_proj_tr(tr)

        skip = cfg.get("skip", set())
        # ---- attention (+ interleaved V-proj b1-chunks / O-proj b0-chunks,
        #      which keep the PE fed while the ScalarE runs the exps) ----
        wo_sb = []

        def load_wo():
            # xT8 tags are dead only once every V-proj chunk has run
            for c2 in range(KD2):
                t = slot(f"xT8{c2}", 2 * D_, FP8)
                dma.dma_start(out=t, in_=wo_d[c2])
                wo_sb.append(t)
        oT8_sb = [slot(f"wq{c2}", 2 * T_, FP8) for c2 in range(KD2)]
        oTu_sb = [slot(f"oTu{hc}", T_, BF16) for hc in range(KD)]
        den_d = dramp.tile([H_, T_], F32, tag="den_d", name="den_d")
        rb_sb = {}

        def attn_scores(b, h):
            hc, par = h // HPC, h % HPC
            at_tiles = [attnp.tile([P, 2 * S_], FP8, tag="attnT",
                                   name="attnT") for _ in range(SKC2)]
            kTm = kTe_sb if par == 0 else kTo_sb
            for kc in range(SKC):
                pt = pall.tile([P, MM_N], F32, tag=f"sc{kc}", name="ps_s")
                nc.tensor.matmul(
                    pt[:, :S_],
                    lhsT=kTm[hc][:, b * S_ + kc * P:b * S_ + (kc + 1) * P],
                    rhs=qT_sb[hc][:, b * S_:(b + 1) * S_],
                    start=True, stop=True)
                bias = (mask2_sb[:, b * SKC + kc:b * SKC + kc + 1]
                        if cfg["has_mask"] else attb_t)
                nc.scalar.activation(
                    out=at_tiles[kc // 2][:, (kc % 2) * S_:(kc % 2 + 1) * S_],
                    in_=pt[:, :S_],
                    func=mybir.ActivationFunctionType.Exp,
                    bias=bias, scale=EXP_SCALE)
            return at_tiles

        def attn_av(b, h, at_tiles):
            hc, par = h // HPC, h % HPC
            pv = pall.tile([P, MM_N], F32, tag=f"pv{h % 2}", name="ps_v2")
            for kc2 in range(SKC2):
                nc.tensor.matmul(
                    pv[0:VW, :S_],
                    lhsT=pair(Vp8_sb[b * SKC2 + kc2],
                              slice(h * VW, (h + 1) * VW)),
                    rhs=pair(at_tiles[kc2], None),
                    start=(kc2 == 0), stop=(kc2 == SKC2 - 1),
                    perf_mode=DR)
            ho = par * HD_
            nc.vector.tensor_copy(
                out=oTu_sb[hc][ho:ho + HD_, b * S_:(b + 1) * S_],
                in_=pv[0:HD_, :S_])
            rs = attn2.tile([1, S_], F32, tag="rs", name="rs")
            nc.vector.tensor_copy(out=rs, in_=pv[HD_:VW, :S_])
            dma2.dma_start(out=den_d[h:h + 1, b * S_:(b + 1) * S_], in_=rs)

        def rb_load(b, hc):
            # broadcast this chunk's denominators back + reciprocal
            sl = slice(b * S_, (b + 1) * S_)
            rb = rbp.tile([P, S_], F32, tag=f"rb{hc}", name="rb")
            rb_sb[hc] = rb
            for h2 in range(HPC):
                dma2.dma_start(
                    out=rb[h2 * HD_:(h2 + 1) * HD_, :],
                    in_=den_d[2 * hc + h2:2 * hc + h2 + 1, sl]
                    .to_broadcast([HD_, S_]))
            nc.vector.reciprocal_approx_fast(out=rb, in_=rb)

        def post_b(b, eng=None):
            # normalize: oT8 = oTu * rb (same partitions). b0 runs on the
            # latency-tolerant GpSimd so the attention vector queue stays
            # clear; b1 runs on vector after the attention phase.
            eng = eng or (nc.gpsimd if b == 0 else nc.vector)
            sl = slice(b * S_, (b + 1) * S_)
            for hc in range(KD):
                eng.tensor_mul(
                    out=oT8_sb[hc // 2][:, (hc % 2) * T_ + b * S_:
                                        (hc % 2) * T_ + (b + 1) * S_],
                    in0=oTu_sb[hc][:, sl], in1=rb_sb[hc])

        # O-proj machinery (defined early so tr 0..3 interleave into b1)
        xm_bf = {}
        xmT8_sb = [slot(f"wk{c2}", 2 * T_, FP8) for c2 in range(KD2)]

        def layer_norm(src, dst, g_bc, b_bc):
            bw = min(512, D_)
            nsub = _ceil_div(D_, bw)
            st = statp.tile([P, nsub, 6], F32, tag="bnst", name="bnst")
            for i in range(nsub):
                nc.vector.bn_stats(out=st[:, i, :],
                                   in_=src[:, i * bw:(i + 1) * bw])
            mv = statp.tile([P, 2], F32, tag="bnmv", name="bnmv")
            nc.vector.bn_aggr(out=mv, in_=st)
            rstd = statp.tile([P, 1], F32, tag="rstd", name="rstd")
            nc.scalar.activation(out=rstd, in_=mv[:, 1:2],
                                 func=mybir.ActivationFunctionType.Sqrt,
                                 bias=eps_t, scale=1.0)
            nc.vector.reciprocal(out=rstd, in_=rstd)
            if g_bc is None:
                nc.vector.tensor_scalar(
                    out=dst, in0=src, scalar1=mv[:, 0:1], scalar2=rstd,
                    op0=mybir.AluOpType.subtract, op1=mybir.AluOpType.mult)
            else:
                tmp = statp.tile([P, D_], F32, tag="lntmp", name="lntmp")
                nc.vector.tensor_scalar(
                    out=tmp, in0=src, scalar1=mv[:, 0:1], scalar2=rstd,
                    op0=mybir.AluOpType.subtract, op1=mybir.AluOpType.mult)
                nc.vector.tensor_mul(out=tmp, in0=tmp, in1=g_bc)
                nc.vector.tensor_add(out=dst, in0=tmp, in1=b_bc)

        def o_proj_tr(tr):
            xt = xrp.tile([P, D_], BF16, tag="xrt", name="xrt")
            dma.dma_start(out=xt, in_=xr_d[tr * P:(tr + 1) * P, :])
            of = work.tile([P, D_], F32, tag="acc", name="of")
            for n2 in range(ND):
                pt = mm_tile()
                for c2 in range(KD2):
                    nc.tensor.matmul(
                        pt[:, :NDW],
                        lhsT=pair(oT8_sb[c2], slice(tr * P, (tr + 1) * P)),
                        rhs=pair(wo_sb[c2],
                                 slice(n2 * MM_N, n2 * MM_N + NDW)),
                        start=(c2 == 0), stop=(c2 == KD2 - 1),
                        perf_mode=DR)
                nc.vector.tensor_add(out=of[:, n2 * MM_N:n2 * MM_N + NDW],
                                     in0=pt[:, :NDW],
                                     in1=xt[:, n2 * MM_N:n2 * MM_N + NDW])
                if cfg["has_bo"]:
                    nc.vector.tensor_add(
                        out=of[:, n2 * MM_N:n2 * MM_N + NDW],
                        in0=of[:, n2 * MM_N:n2 * MM_N + NDW],
                        in1=bo_bc[:, n2 * MM_N:n2 * MM_N + NDW])
            xm = slot(f"qT{tr}", D_, BF16)   # reuse qT slot (scores done)
            xm_bf[tr] = xm
            if "ln" in skip:
                nc.vector.tensor_copy(out=xm, in_=of)
            else:
                layer_norm(of, xm,
                           g1_bc if cfg["has_n1"] else None,
                           b1_bc if cfg["has_n1"] else None)

        def transpose_tr(tr):
            for c in range(KD):
                if "tr" in skip:
                    nc.vector.tensor_copy(
                        out=xmT8_sb[c // 2][:, (c % 2) * T_ + tr * P:
                                            (c % 2) * T_ + (tr + 1) * P],
                        in_=xm_bf[tr][:, c * P:(c + 1) * P])
                else:
                    pt = pall.tile([P, P], BF16, tag=f"sc{c % 4}",
                                   name="ps_t")
                    nc.tensor.transpose(pt, xm_bf[tr][:, c * P:(c + 1) * P],
                                        ident)
                    nc.vector.tensor_copy(
                        out=xmT8_sb[c // 2][:, (c % 2) * T_ + tr * P:
                                            (c % 2) * T_ + (tr + 1) * P],
                        in_=pt)

        if "attn" in skip:
            for c in range(KD):
                nc.vector.tensor_copy(
                    out=oT8_sb[c // 2][:, (c % 2) * T_:(c % 2 + 1) * T_],
                    in_=qT_sb[c])
            for tr in range(TCH // 2, TCH):
                v_proj_tr(tr)
            for m in range(KD):
                qk_proj("q", m, 1)
                qk_proj("k", m, 1)
            load_wo()
            for tr in range(TCH):
                o_proj_tr(tr)
                if tr > 0:
                    transpose_tr(tr - 1)
            transpose_tr(TCH - 1)
        else:
            # PE fillers: b0 gets the b1-token V chunks + first deferred
            # Q/K halves; b1 gets the remaining deferred Q/K halves (their
            # evicts are vector-only, so the exp stream never stalls).
            fill_b0 = {2: ("v", TCH // 2), 5: ("v", TCH // 2 + 1),
                       8: ("v", TCH // 2 + 2), 11: ("v", TCH // 2 + 3),
                       13: ("qk", 0), 14: ("qk", 1)}
            fill_b1 = {0: ("qk", 2), 2: ("qk", 3), 4: ("qk", 4),
                       6: ("qk", 5), 8: ("qk", 6), 10: ("qk", 7)}
            prev = None
            for b in range(TB_):
                fills = fill_b0 if b == 0 else fill_b1
                for h in range(H_):
                    at = attn_scores(b, h)
                    if prev is not None:
                        attn_av(*prev)
                        if prev[1] % 2 == 1:
                            rb_load(prev[0], prev[1] // 2)
                        if prev[:2] == (1, 3):
                            post_b(0)
                    prev = (b, h, at)
                    if h in fills:
                        kind, arg = fills[h]
                        if kind == "v":
                            v_proj_tr(arg)
                        else:
                            qk_proj("q", arg, 1)
                            qk_proj("k", arg, 1)
                    if b == 1 and h == 10:
                        load_wo()
                if b == 0:
                    attn_av(*prev)
                    rb_load(0, KD - 1)
                    prev = None
            attn_av(*prev)
            rb_load(1, KD - 1)
            # O-proj for b0 token chunks (oT8 b0 half ready via post_b(0));
            # LN1 sqrts now run after the last exp -- no act-table churn.
            for tr in range(TCH // 2):
                o_proj_tr(tr)
                if tr > 0:
                    transpose_tr(tr - 1)
            post_b(1)
            transpose_tr(TCH // 2 - 1)
            for tr in range(TCH // 2, TCH):
                o_proj_tr(tr)
                transpose_tr(tr - 1)
            transpose_tr(TCH - 1)

        # ---- FFN up weights (after all kTe/kTo readers; tag reuse) ----
        up_tags = [t for c in range(KD) for t in (f"kTe{c}", f"kTo{c}")]
        wup_sb = []
        for i in range(KD2 * UPT):
            t = slot(up_tags[i], 2 * UPW, FP8)
            dma.dma_start(out=t, in_=wup_d[i])
            wup_sb.append(t)

        def wup_lhsT(c2, fm):
            i = c2 * UPT + (fm * P) // UPW
            o = (fm * P) % UPW
            return pair(wup_sb[i], slice(o, o + P))

        # ---- FFN up (fp8 DR) + Gelu -> gT ----
        g_tags = ([f"g{c}" for c in range(KF - KD - KD2)]
                  + [f"wv{c2}" for c2 in range(KD2)]
                  + [f"oTu{hc}" for hc in range(KD)])
        gT_sb = [slot(g_tags[c], T_, BF16) for c in range(KF)]
        for fm in range(KF):
            for t2 in range(NT):
                pt = mm_tile()
                for c2 in range(KD2):
                    nc.tensor.matmul(
                        pt[:, :NTW],
                        lhsT=wup_lhsT(c2, fm),
                        rhs=pair(xmT8_sb[c2],
                                 slice(t2 * MM_N, t2 * MM_N + NTW)),
                        start=(c2 == 0), stop=(c2 == KD2 - 1),
                        perf_mode=DR)
                nc.scalar.activation(
                    out=gT_sb[fm][:, t2 * MM_N:t2 * MM_N + NTW],
                    in_=pt[:, :NTW],
                    func=mybir.ActivationFunctionType.Gelu,
                    bias=(bup_sb[:, fm:fm + 1] if cfg["has_bup"]
                          else zero_t),
                    scale=1.0 / WS)

        # ---- FFN down (bf16) + residual + LN2 -> out ----
        dn_tags = ([f"Vp{c}" for c in range(TCH // 2)]
                   + [f"xT8{c2}" for c2 in range(KD2)]
                   + [f"wq{c2}" for c2 in range(KD2)]
                   + up_tags
                   + [f"wk{c2}" for c2 in range(KD2)])
        wdn_sb = []
        for fc in range(KF):
            t = slot(dn_tags[fc], D_, BF16)
            dma.dma_start(out=t, in_=wdn_d[fc * P:(fc + 1) * P, :])
            wdn_sb.append(t)
        for tr in range(TCH):
            dsb = work.tile([P, D_], F32, tag="acc", name="dsb")
            for n2 in range(ND):
                pt = mm_tile()
                for fc in range(KF):
                    nc.tensor.matmul(
                        pt[:, :NDW],
                        lhsT=gT_sb[fc][:, tr * P:(tr + 1) * P],
                        rhs=wdn_sb[fc][:, n2 * MM_N:n2 * MM_N + NDW],
                        start=(fc == 0), stop=(fc == KF - 1))
                nc.vector.tensor_add(
                    out=dsb[:, n2 * MM_N:n2 * MM_N + NDW],
                    in0=pt[:, :NDW],
                    in1=xm_bf[tr][:, n2 * MM_N:n2 * MM_N + NDW])
                if cfg["has_bdn"]:
                    nc.vector.tensor_add(
                        out=dsb[:, n2 * MM_N:n2 * MM_N + NDW],
                        in0=dsb[:, n2 * MM_N:n2 * MM_N + NDW],
                        in1=bdn_bc[:, n2 * MM_N:n2 * MM_N + NDW])
            ot = outp.tile([P, D_], F32, tag="ot", name="ot")
            if "ln" in skip:
                nc.vector.tensor_copy(out=ot, in_=dsb)
            else:
                layer_norm(dsb, ot,
                           g2_bc if cfg["has_n2"] else None,
                           b2_bc if cfg["has_n2"] else None)
            if tr < TCH - 1:
                # late chunks avoid the slow-to-drain software (gpsimd) queue
                hw = D_ // 2
                dma.dma_start(out=out_d[tr * P:(tr + 1) * P, 0:hw],
                              in_=ot[:, 0:hw])
                (dma2 if tr < TCH // 2 else nc.scalar).dma_start(
                    out=out_d[tr * P:(tr + 1) * P, hw:D_], in_=ot[:, hw:D_])
            else:
                # last chunk is latency-exposed: split across HW queues
                qw = D_ // 4
                engs = (nc.sync, nc.scalar, nc.sync, nc.scalar)
                for qi, eng in enumerate(engs):
                    eng.dma_start(
                        out=out_d[tr * P:(tr + 1) * P,
                                  qi * qw:(qi + 1) * qw],
                        in_=ot[:, qi * qw:(qi + 1) * qw])

    nc.finalize()
    return nc


_PROGRAM_CACHE = {}


def _get_program(cfg_key, cfg):
    if cfg_key not in _PROGRAM_CACHE:
        _PROGRAM_CACHE[cfg_key] = build_program(cfg)
    return _PROGRAM_CACHE[cfg_key]


def _swz(w, npairs, width):
    """[rows, cols] -> [npairs, 128, 2*cols] K-paired contiguous."""
    return np.ascontiguousarray(
        w.reshape(npairs, 2, P, width).transpose(0, 2, 1, 3)
        .reshape(npairs, P, 2 * width))


def make_in_maps(inputs):
    f32 = np.float32
    x = np.asarray(inputs["x"], f32)
    scale = 1.0 / np.sqrt(float(inputs["head_dim"]))

    def merged(w, a, b):
        return (np.asarray(w, f32)
                + np.asarray(a, f32) @ np.asarray(b, f32))

    KD2 = D // P // 2
    wq = _swz((merged(inputs["w_q"], inputs["w_q_lora_a"],
                      inputs["w_q_lora_b"]) * (scale * QS)).astype(NP_FP8),
              KD2, D)
    wk = _swz((merged(inputs["w_k"], inputs["w_k_lora_a"],
                      inputs["w_k_lora_b"]) * WS).astype(NP_FP8), KD2, D)
    wv = _swz((merged(inputs["w_v"], inputs["w_v_lora_a"],
                      inputs["w_v_lora_b"]) * WS).astype(NP_FP8), KD2, D)
    wo = _swz((merged(inputs["w_o"], inputs["w_o_lora_a"],
                      inputs["w_o_lora_b"]) * WS).astype(NP_FP8), KD2, D)
    wup8 = (merged(inputs["w_up"], inputs["w_up_lora_a"],
                   inputs["w_up_lora_b"]) * WS).astype(NP_FP8)
    UPW = 1024
    UPT = F // UPW
    wup = np.ascontiguousarray(
        wup8.reshape(KD2, 2, P, UPT, UPW).transpose(0, 3, 2, 1, 4)
        .reshape(KD2 * UPT, P, 2 * UPW))
    wdn = merged(inputs["w_down"], inputs["w_down_lora_a"],
                 inputs["w_down_lora_b"]).astype(NP_BF16)
    mask = np.asarray(inputs["attention_mask"], f32)

    common = {
        "wq": wq, "wk": wk, "wv": wv, "wo": wo, "wup": wup, "wdn": wdn,
        "bq": (np.asarray(inputs["b_q"], f32) * (scale * QS)).astype(f32),
        "bk": (np.asarray(inputs["b_k"], f32) * WS).astype(f32),
        "bup": np.asarray(inputs["b_up"], f32),
        "bv": np.asarray(inputs["b_v"], f32),
        "bo": np.asarray(inputs["b_o"], f32),
        "bdn": np.asarray(inputs["b_down"], f32),
        "g1": np.asarray(inputs["norm_weight_1"], f32),
        "b1": np.asarray(inputs["norm_bias_1"], f32),
        "g2": np.asarray(inputs["norm_weight_2"], f32),
        "b2": np.asarray(inputs["norm_bias_2"], f32),
    }
    in_maps = []
    for i in range(N_CORES):
        xc = x[i * TB:(i + 1) * TB].reshape(T, D)
        m = dict(common)
        m["xT8"] = _swz(np.ascontiguousarray(xc.T).astype(NP_FP8), KD2, T)
        m["xr"] = (np.ascontiguousarray(xc) * WS).astype(NP_BF16)
        m["maskT"] = np.ascontiguousarray(mask[i * TB:(i + 1) * TB, 0, 0, :])
        in_maps.append(m)
    return in_maps


def full_cfg(inputs):
    f32 = np.float32
    return {
        "D": D, "F": F, "T": T, "TB": TB, "H": H, "HD": HD,
        "has_bq": bool(np.any(np.asarray(inputs["b_q"], f32))),
        "has_bk": bool(np.any(np.asarray(inputs["b_k"], f32))),
        "has_bup": bool(np.any(np.asarray(inputs["b_up"], f32))),
        "has_mask": bool(np.any(np.asarray(inputs["attention_mask"], f32))),
        "has_bv": bool(np.any(np.asarray(inputs["b_v"], f32))),
        "has_bo": bool(np.any(np.asarray(inputs["b_o"], f32))),
        "has_bdn": bool(np.any(np.asarray(inputs["b_down"], f32))),
        "has_n1": bool(np.any(np.asarray(inputs["norm_weight_1"], f32) != 1.0)
                       or np.any(np.asarray(inputs["norm_bias_1"], f32))),
        "has_n2": bool(np.any(np.asarray(inputs["norm_weight_2"], f32) != 1.0)
                       or np.any(np.asarray(inputs["norm_bias_2"], f32))),
    }


def run_on_hw(inputs, trace=False, tmpdir=None):
    cfg = full_cfg(inputs)
    cfg_key = tuple(sorted((k, v) for k, v in cfg.items()
                           if not isinstance(v, set)))
    nc = _get_program(cfg_key, cfg)
    in_maps = make_in_maps(inputs)
    kw = {}
    if trace:
        kw = {"trace": True, "tmpdir": tmpdir}
    res = run_bass_kernel_spmd(nc, in_maps, core_ids=list(range(N_CORES)),
                               **kw)
    out = np.empty((B, S, D), np.float32)
    for i in range(N_CORES):
        out[i * TB:(i + 1) * TB] = res.results[i]["out"].reshape(TB, S, D)
    return out, res


def kernel(**inputs):
    out, _ = run_on_hw(inputs)
    return out
